# revision 20
# baseline (speedup 1.0000x reference)
"""Neural CDE forward pass on 8 Trainium2 NeuronCores.

Model (reference): z0 = coeffs[:,0]@W_init+b_init; RK4 scan over T-1=99 grid
intervals of dz = f(z) dX with f = MLP(64->128->128->128->512) -> tanh ->
reshape [H,C], contracted with dX/dt; then logits/loss/accuracy readout.

Sharding: pure data parallel over batch (2048 -> 8x256). Each core runs the
full scan on its shard; tiny readout done on host from the final z.

Device layout (per core): the 256-row batch shard is split into 2 independent
128-wide streams. All matmul operands are fp16 (1 cyc/row on PE); PSUM
accumulates fp32; the carried z state stays fp32. The wall-clock is set by
the serial RK4 dependency chain (396 stages), so the chain is shortened:

  - G-trick: the next stage's first-layer preactivation is computed directly
    in PSUM as  W_in^T z  (seed matmul)  +  a*dt*(S_fold W_in)^T prod
    (4 chunk matmuls), eliminating the fold->zs->mm1 round trip.
  - The RK4 combination  z' = z + sum_s w_s k_s  accumulates in a PSUM bank
    via 16 fold matmuls with pre-scaled stationaries (w_s * S_fold), read
    once per step.
  - [128->512] output layer: 4 column chunks (permuted so chunk j /
    partition p holds feature (h=p%64, c=2j+(p>=64))), bias via a K=4 seed
    matmul, tanh on Act, dX multiply on DVE (dX, not dX/dt: the dt factors
    are folded into the fold stationaries so all fp16 constants stay O(1)).
  - A second PSUM accumulator (pv) carries W_in^T z_{t+1} across the step
    boundary (seed + w_s-scaled G folds), so the step handoff is a normal
    stage edge; the fp32 z update runs off-chain on DVE.
  - pv and pz share one PSUM bank: a matmul with start=True resets the
    WHOLE bank (measured on HW), so the bank sees exactly one start per
    step (the pv seed, which also zeroes pz) and everything else
    accumulates with start=False.
  - The w_s/a_s-weighted fold trains are deferred past the next stage's
    mm3 emission so the in-order PE runs them inside the relu wait windows.
  - relu1/relu3 + prod on DVE, relu2 + tanh on Act (gpsimd cannot touch
    PSUM on TRN2).
"""

import numpy as np

from contextlib import ExitStack

from concourse import bacc, mybir
import concourse.tile as tile
from concourse.bass_utils import run_bass_kernel_spmd

N_CORES = 8
B, T, C, H, HH, O = 2048, 100, 8, 64, 128, 10
BS = B // N_CORES   # 256 batch rows per core
SW = 128            # stream width; 2 streams per core
F16 = mybir.dt.float16
F32 = mybir.dt.float32

ADD = mybir.AluOpType.add
MAX = mybir.AluOpType.max
MULT = mybir.AluOpType.mult
TANH = mybir.ActivationFunctionType.Tanh
RELU = mybir.ActivationFunctionType.Relu


def _build(nsteps, dts):
    """Build + compile the per-core Bass program. dts: python floats [nsteps],
    must be uniform (linspace grid)."""
    nc = bacc.Bacc("TRN2", target_bir_lowering=False, debug=False,
                   num_devices=N_CORES)

    def din(name, shape, dt=F16):
        return nc.dram_tensor(name, shape, dt, kind="ExternalInput").ap()

    z32_d = din("z32", [2, H, SW], F32)
    z16_d = din("z16", [2, H, SW], F16)
    drep_d = din("drep", [nsteps * 2, 128, 512], F16)
    w_in_d = din("w_in", [H, HH])
    w_h0_d = din("w_h0", [HH, HH])
    w_h1_d = din("w_h1", [HH, HH])
    w_out_d = din("w_out", [HH, 4 * HH])
    g_half_d = din("g_half", [HH, HH])
    g_full_d = din("g_full", [HH, HH])
    sw6_d = din("sw6", [HH, H])
    sw3_d = din("sw3", [HH, H])
    gw6_d = din("gw6", [HH, HH])
    gw3_d = din("gw3", [HH, HH])
    bias4_d = din("bias4", [4, HH])
    ind4_d = din("ind4", [4, 512])
    b_in_d = din("b_in", [HH, 1], F32)
    b_h0_d = din("b_h0", [HH, 1], F32)
    b_h1_d = din("b_h1", [HH, 1], F32)
    zT_d = nc.dram_tensor("zT", [2, H, SW], F32, kind="ExternalOutput").ap()

    with tile.TileContext(nc) as tc, ExitStack() as ctx:
        const = ctx.enter_context(tc.tile_pool(name="const", bufs=1))

        def load(ap_dram, shape, dt=F16):
            t = const.tile(shape, dt, tag=ap_dram.name, name=ap_dram.name)
            nc.sync.dma_start(t[:], ap_dram)
            return t

        w_in = load(w_in_d, [H, HH])
        w_h0 = load(w_h0_d, [HH, HH])
        w_h1 = load(w_h1_d, [HH, HH])
        w_out = load(w_out_d, [HH, 4 * HH])
        g_half = load(g_half_d, [HH, HH])
        g_full = load(g_full_d, [HH, HH])
        sw6 = load(sw6_d, [HH, H])
        sw3 = load(sw3_d, [HH, H])
        gw6 = load(gw6_d, [HH, HH])
        gw3 = load(gw3_d, [HH, HH])
        bias4 = load(bias4_d, [4, HH])
        ind4 = load(ind4_d, [4, 512])
        b_in = load(b_in_d, [HH, 1], F32)
        b_h0 = load(b_h0_d, [HH, 1], F32)
        b_h1 = load(b_h1_d, [HH, 1], F32)

        sb_h = ctx.enter_context(tc.tile_pool(name="h", bufs=3))
        sb_f = ctx.enter_context(tc.tile_pool(name="f", bufs=2))
        sb_p = ctx.enter_context(tc.tile_pool(name="prod", bufs=2))
        sb_z16 = ctx.enter_context(tc.tile_pool(name="z16p", bufs=2))
        sb_z32 = ctx.enter_context(tc.tile_pool(name="z32p", bufs=2))
        sb_d = ctx.enter_context(tc.tile_pool(name="d", bufs=4))
        ps_h = ctx.enter_context(tc.tile_pool(name="psh", bufs=2, space="PSUM"))
        ps_mix = ctx.enter_context(tc.tile_pool(name="psmix", bufs=1, space="PSUM"))
        ps_f = ctx.enter_context(tc.tile_pool(name="psf", bufs=1, space="PSUM"))

        z32 = [None, None]
        z16 = [None, None]
        for s in range(2):
            z32[s] = const.tile([H, SW], F32, tag=f"z32i{s}", name=f"z32i{s}")
            nc.sync.dma_start(z32[s][:], z32_d[s])
            z16[s] = const.tile([H, SW], F16, tag=f"z16i{s}", name=f"z16i{s}")
            nc.sync.dma_start(z16[s][:], z16_d[s])

        # Per-stream packed accumulator bank (ring of 2):
        #   cols 0:SW           -> pv: W_in^T z_next accumulator
        #   cols SW:2SW (64 prt)-> pz: sum_s w_s k_s accumulator
        mix = [None, None]
        prev_pv = [None, None]

        # initial pv = W_in^T z0
        for s in range(2):
            mix[s] = ps_mix.tile([128, 2 * SW], F32, tag=f"mix{s}",
                                 name=f"mixi{s}")
            nc.tensor.matmul(mix[s][:, 0:SW], w_in[:], z16[s][:],
                             start=True, stop=True)
            prev_pv[s] = mix[s][:, 0:SW]

        def ph_alloc(s):
            t = ps_h.tile([HH, SW], F32, tag=f"ph{s}", name=f"ph{s}")
            return t[:]

        for ti in range(nsteps):
            d_t = [None, None]
            for s in range(2):
                d_t[s] = sb_d.tile([128, 512], F16, tag=f"d{s}", name=f"d{s}")
                nc.sync.dma_start(d_t[s][:], drep_d[2 * ti + s])

            for s in range(2):
                mix[s] = ps_mix.tile([128, 2 * SW], F32, tag=f"mix{s}",
                                     name=f"mix{s}")
            pz = [mix[0][0:H, SW:2 * SW], mix[1][0:H, SW:2 * SW]]
            pv = [mix[0][:, 0:SW], mix[1][:, 0:SW]]
            ph1 = [prev_pv[0], prev_pv[1]]

            deferred = []
            for s4 in range(4):
                h1 = [None, None]
                for s in range(2):
                    h1[s] = sb_h.tile([HH, SW], F16, tag=f"h1{s}", name=f"h1{s}")
                    nc.vector.tensor_scalar(h1[s][:], ph1[s], b_in[:], 0.0,
                                            op0=ADD, op1=MAX)
                if s4 == 0:
                    # seed pv accumulator: W_in^T z_t (z16 of this step)
                    for s in range(2):
                        nc.tensor.matmul(pv[s], w_in[:], z16[s][:],
                                         start=True, stop=False,
                                         skip_group_check=True)
                ph2 = [None, None]
                for s in range(2):
                    ph2[s] = ph_alloc(s)
                    nc.tensor.matmul(ph2[s], w_h0[:], h1[s][:],
                                     start=True, stop=True)
                h2 = [None, None]
                for s in range(2):
                    h2[s] = sb_h.tile([HH, SW], F16, tag=f"h2{s}", name=f"h2{s}")
                    nc.scalar.activation(h2[s][:], ph2[s], RELU,
                                         bias=b_h0[:, 0:1])
                ph3 = [None, None]
                for s in range(2):
                    ph3[s] = ph_alloc(s)
                    nc.tensor.matmul(ph3[s], w_h1[:], h2[s][:],
                                     start=True, stop=True)
                for emit in deferred:
                    emit()
                deferred = []
                h3 = [None, None]
                for s in range(2):
                    h3[s] = sb_h.tile([HH, SW], F16, tag=f"h3{s}", name=f"h3{s}")
                    nc.vector.tensor_scalar(h3[s][:], ph3[s], b_h1[:], 0.0,
                                            op0=ADD, op1=MAX)
                pf = [None, None]
                for s in range(2):
                    pf[s] = ps_f.tile([128, 512], F32, tag=f"pf{s}", name=f"pf{s}")
                    nc.tensor.matmul(pf[s][:], bias4[:], ind4[:],
                                     start=True, stop=False,
                                     skip_group_check=True)
                for s in range(2):
                    for j in range(4):
                        nc.tensor.matmul(pf[s][:, 128 * j:128 * (j + 1)],
                                         w_out[:, 128 * j:128 * (j + 1)],
                                         h3[s][:],
                                         start=False, stop=(j == 3),
                                         skip_group_check=True)
                f_sb = [None, None]
                for s in range(2):
                    f_sb[s] = sb_f.tile([128, 512], F16, tag=f"f{s}", name=f"f{s}")
                for s in range(2):
                    nc.scalar.activation(f_sb[s][:], pf[s][:], TANH)
                prod = [None, None]
                for s in range(2):
                    prod[s] = sb_p.tile([128, 512], F16, tag=f"pr{s}", name=f"pr{s}")
                for s in range(2):
                    nc.vector.tensor_tensor(prod[s][:], f_sb[s][:],
                                            d_t[s][:], op=MULT)

                if s4 < 3:
                    g_mat = g_half if s4 < 2 else g_full
                    nxt = [None, None]
                    for s in range(2):
                        nxt[s] = ph_alloc(s)
                        nc.tensor.matmul(nxt[s], w_in[:], z16[s][:],
                                         start=True, stop=False,
                                         skip_group_check=True)
                    for s in range(2):
                        for j in range(4):
                            nc.tensor.matmul(nxt[s], g_mat[:],
                                             prod[s][:, 128 * j:128 * (j + 1)],
                                             start=False, stop=(j == 3),
                                             skip_group_check=True)
                    ph1 = [nxt[0], nxt[1]]
                else:
                    # boundary: pv must close before next step's relu1
                    for s in range(2):
                        for j in range(4):
                            nc.tensor.matmul(pv[s], gw6[:],
                                             prod[s][:, 128 * j:128 * (j + 1)],
                                             start=False, stop=(j == 3),
                                             skip_group_check=True)

                # off-chain accumulators: defer past the next stage's mm3 so
                # the in-order PE chews them inside the relu2/relu3 windows
                def make_folds(s4, prod):
                    def emit():
                        sw_mat = sw6 if s4 in (0, 3) else sw3
                        for s in range(2):
                            for j in range(4):
                                # the pv seed's start=True already zeroed
                                # this bank (incl. the pz region): accumulate
                                nc.tensor.matmul(
                                    pz[s], sw_mat[:],
                                    prod[s][:, 128 * j:128 * (j + 1)],
                                    start=False,
                                    stop=(s4 == 3 and j == 3),
                                    skip_group_check=True)
                        if s4 < 3:
                            gw_mat = gw6 if s4 in (0, 3) else gw3
                            for s in range(2):
                                for j in range(4):
                                    nc.tensor.matmul(
                                        pv[s], gw_mat[:],
                                        prod[s][:, 128 * j:128 * (j + 1)],
                                        start=False, stop=False,
                                        skip_group_check=True)
                    return emit

                if s4 < 3:
                    deferred.append(make_folds(s4, prod))
                else:
                    make_folds(s4, prod)()

            # step end (off the critical chain): z' = z + pz
            for s in range(2):
                z16_n = sb_z16.tile([H, SW], F16, tag=f"z16{s}", name=f"z16{s}")
                nc.vector.tensor_tensor(z16_n[:], pz[s], z32[s][:], op=ADD)
                z16[s] = z16_n
            for s in range(2):
                z32_n = sb_z32.tile([H, SW], F32, tag=f"z32{s}", name=f"z32{s}")
                nc.vector.tensor_tensor(z32_n[:], pz[s], z32[s][:], op=ADD)
                z32[s] = z32_n
            prev_pv = [pv[0], pv[1]]

        for s in range(2):
            nc.sync.dma_start(zT_d[s], z32[s][:])

    nc.compile()
    return nc


def _prep_inputs(coeffs, times, W_init, b_init, W_in, b_in, W_h, b_h,
                 W_out, b_out, nsteps):
    """Host-side constants + per-core shards."""
    coeffs = np.asarray(coeffs, np.float32)
    times = np.asarray(times, np.float32)
    dts_full = np.diff(times)
    dx = coeffs[:, 1:, :] - coeffs[:, :-1, :]
    dts = dts_full[:nsteps]
    dx = dx[:, :nsteps, :]
    dt0 = float(dts[0])
    assert np.allclose(dts, dt0, rtol=1e-4), "kernel assumes a uniform grid"

    z0 = coeffs[:, 0, :] @ np.asarray(W_init, np.float32) + np.asarray(b_init, np.float32)
    z0 = np.ascontiguousarray(z0.T)                      # [H, B] f32

    p = np.arange(128)
    j = np.arange(4)
    c_idx = 2 * j[None, :] + (p[:, None] >= 64)          # [128, 4]
    col = (p[:, None] % 64) * 8 + c_idx                  # [128, 4] output col

    W_in_f = np.asarray(W_in, np.float32)                # [H, HH]
    W_out = np.asarray(W_out, np.float32)                # [HH, 512]
    b_out = np.asarray(b_out, np.float32)                # [512]
    w_out_perm = np.ascontiguousarray(
        W_out[:, col.T.reshape(-1)]).astype(np.float16)  # [HH, (j,p) 512]
    bias4 = np.ascontiguousarray(b_out[col.T]).astype(np.float16)  # [4, 128]
    ind4 = np.kron(np.eye(4), np.ones((1, 128))).astype(np.float16)

    s_fold = (p[:, None] % 64 == np.arange(H)[None, :]).astype(np.float32)
    # G matrices: G_a[p, m] = a*dt * W_in[p%64, m]  (= a*dt * (S_fold @ W_in))
    g_base = s_fold @ W_in_f                             # [128, HH]
    g_half = (0.5 * g_base).astype(np.float16)
    g_full = (1.0 * g_base).astype(np.float16)
    sw6 = (s_fold / 6.0).astype(np.float16)
    sw3 = (s_fold / 3.0).astype(np.float16)
    gw6 = (g_base / 6.0).astype(np.float16)
    gw3 = (g_base / 3.0).astype(np.float16)

    W_h = np.asarray(W_h, np.float32)
    b_h = np.asarray(b_h, np.float32)
    consts = {
        "w_in": W_in_f.astype(np.float16),
        "w_h0": W_h[0].astype(np.float16),
        "w_h1": W_h[1].astype(np.float16),
        "w_out": w_out_perm,
        "g_half": g_half, "g_full": g_full,
        "sw6": sw6, "sw3": sw3,
        "gw6": gw6, "gw3": gw3,
        "bias4": bias4, "ind4": ind4,
        "b_in": np.asarray(b_in, np.float32).reshape(HH, 1).copy(),
        "b_h0": b_h[0].reshape(HH, 1).copy(),
        "b_h1": b_h[1].reshape(HH, 1).copy(),
    }

    in_maps = []
    for ci in range(N_CORES):
        bs, be = ci * BS, (ci + 1) * BS
        dx_t = dx[bs:be].transpose(1, 2, 0)              # [nsteps, C, 256]
        arr = dx_t.reshape(nsteps, 4, 2, 2, SW)          # [t, j, q, s, b]
        arr2 = arr.transpose(0, 3, 2, 1, 4).astype(np.float16)
        drep = np.broadcast_to(
            arr2[:, :, :, None, :, :],
            (nsteps, 2, 2, 64, 4, SW)).reshape(nsteps * 2, 128, 512)
        m = dict(consts)
        m["drep"] = np.ascontiguousarray(drep)
        zc = z0[:, bs:be]                                # [H, 256]
        m["z32"] = np.ascontiguousarray(
            zc.reshape(H, 2, SW).transpose(1, 0, 2))     # [2, H, SW]
        m["z16"] = m["z32"].astype(np.float16)
        in_maps.append(m)
    return in_maps, dts


_CACHE = {}


def _get_nc(nsteps, dts_key, dts):
    key = (nsteps, dts_key)
    if key not in _CACHE:
        _CACHE[key] = _build(nsteps, dts)
    return _CACHE[key]


def run_scan(coeffs, times, W_init, b_init, W_in, b_in, W_h, b_h, W_out, b_out,
             nsteps=None):
    """Run the device scan; returns zT [B, H] float32."""
    times = np.asarray(times, np.float32)
    if nsteps is None:
        nsteps = len(times) - 1
    in_maps, dts = _prep_inputs(coeffs, times, W_init, b_init, W_in, b_in,
                                W_h, b_h, W_out, b_out, nsteps)
    nc = _get_nc(nsteps, dts.tobytes(), dts)
    res = run_bass_kernel_spmd(nc, in_maps, core_ids=list(range(N_CORES)))
    outs = []
    for ci in range(N_CORES):
        zT = res.results[ci]["zT"]                       # [2, H, SW]
        outs.append(zT.transpose(1, 0, 2).reshape(H, BS))
    zT = np.concatenate(outs, axis=1)                    # [H, B]
    return np.ascontiguousarray(zT.T)


def kernel(coeffs, y, times, W_init, b_init, W_in, b_in, W_h, b_h,
           W_out, b_out, W_read, b_read):
    zT = run_scan(coeffs, times, W_init, b_init, W_in, b_in, W_h, b_h,
                  W_out, b_out)
    y = np.asarray(y)
    logits = (zT.astype(np.float64) @ np.asarray(W_read, np.float64)
              + np.asarray(b_read, np.float64))          # [B, O]
    m = logits.max(axis=1, keepdims=True)
    logp = logits - (m + np.log(np.exp(logits - m).sum(axis=1, keepdims=True)))
    loss = np.float32(-logp[np.arange(B), y].mean())
    acc = np.float32((logits.argmax(axis=1) == y).sum())
    return loss, acc


# revision 23
# speedup vs baseline: 1.0043x; 1.0043x over previous
"""Neural CDE forward pass on 8 Trainium2 NeuronCores.

Model (reference): z0 = coeffs[:,0]@W_init+b_init; RK4 scan over T-1=99 grid
intervals of dz = f(z) dX with f = MLP(64->128->128->128->512) -> tanh ->
reshape [H,C], contracted with dX/dt; then logits/loss/accuracy readout.

Sharding: pure data parallel over batch (2048 -> 8x256). Each core runs the
full scan on its shard; tiny readout done on host from the final z.

Device layout (per core): the 256-row batch shard is split into 2 independent
128-wide streams. All matmul operands are fp16 (1 cyc/row on PE); PSUM
accumulates fp32; the carried z state stays fp32. The wall-clock is set by
the serial RK4 dependency chain (396 stages), so the chain is shortened:

  - G-trick: the next stage's first-layer preactivation is computed directly
    in PSUM as  W_in^T z  (seed matmul)  +  a*dt*(S_fold W_in)^T prod
    (4 chunk matmuls), eliminating the fold->zs->mm1 round trip.
  - The RK4 combination  z' = z + sum_s w_s k_s  accumulates in a PSUM bank
    via 16 fold matmuls with pre-scaled stationaries (w_s * S_fold), read
    once per step.
  - [128->512] output layer: 4 column chunks (permuted so chunk j /
    partition p holds feature (h=p%64, c=2j+(p>=64))), bias via a K=4 seed
    matmul, tanh on Act, dX multiply on DVE (dX, not dX/dt: the dt factors
    are folded into the fold stationaries so all fp16 constants stay O(1)).
  - A second PSUM accumulator (pv) carries W_in^T z_{t+1} across the step
    boundary (seed + w_s-scaled G folds), so the step handoff is a normal
    stage edge; the fp32 z update runs off-chain on DVE.
  - pv and pz share one PSUM bank: a matmul with start=True resets the
    WHOLE bank (measured on HW), so the bank sees exactly one start per
    step (the pv seed, which also zeroes pz) and everything else
    accumulates with start=False.
  - The w_s/a_s-weighted fold trains are deferred past the next stage's
    mm3 emission so the in-order PE runs them inside the relu wait windows.
  - relu1/relu3 + prod on DVE, relu2 + tanh on Act (gpsimd cannot touch
    PSUM on TRN2).
"""

import numpy as np

from contextlib import ExitStack

from concourse import bacc, mybir
import concourse.tile as tile
from concourse.bass_utils import run_bass_kernel_spmd

N_CORES = 8
B, T, C, H, HH, O = 2048, 100, 8, 64, 128, 10
BS = B // N_CORES   # 256 batch rows per core
SW = 128            # stream width; 2 streams per core
F16 = mybir.dt.float16
F32 = mybir.dt.float32

ADD = mybir.AluOpType.add
MAX = mybir.AluOpType.max
MULT = mybir.AluOpType.mult
TANH = mybir.ActivationFunctionType.Tanh
RELU = mybir.ActivationFunctionType.Relu


def _build(nsteps, dts):
    """Build + compile the per-core Bass program. dts: python floats [nsteps],
    must be uniform (linspace grid)."""
    nc = bacc.Bacc("TRN2", target_bir_lowering=False, debug=False,
                   num_devices=N_CORES)

    def din(name, shape, dt=F16):
        return nc.dram_tensor(name, shape, dt, kind="ExternalInput").ap()

    z32_d = din("z32", [2, H, SW], F32)
    z16_d = din("z16", [2, H, SW], F16)
    drep_d = din("drep", [nsteps * 2, 128, 512], F16)
    w_in_d = din("w_in", [H, HH])
    w_h0_d = din("w_h0", [HH, HH])
    w_h1_d = din("w_h1", [HH, HH])
    w_out_d = din("w_out", [HH, 4 * HH])
    g_half_d = din("g_half", [HH, HH])
    g_full_d = din("g_full", [HH, HH])
    sw6_d = din("sw6", [HH, H])
    sw3_d = din("sw3", [HH, H])
    gw6_d = din("gw6", [HH, HH])
    gw3_d = din("gw3", [HH, HH])
    bias4_d = din("bias4", [4, HH])
    ind4_d = din("ind4", [4, 512])
    b_in_d = din("b_in", [HH, 1], F32)
    b_h0_d = din("b_h0", [HH, 1], F32)
    b_h1_d = din("b_h1", [HH, 1], F32)
    zT_d = nc.dram_tensor("zT", [2, H, SW], F32, kind="ExternalOutput").ap()

    with tile.TileContext(nc) as tc, ExitStack() as ctx:
        const = ctx.enter_context(tc.tile_pool(name="const", bufs=1))

        def load(ap_dram, shape, dt=F16):
            t = const.tile(shape, dt, tag=ap_dram.name, name=ap_dram.name)
            nc.sync.dma_start(t[:], ap_dram)
            return t

        w_in = load(w_in_d, [H, HH])
        w_h0 = load(w_h0_d, [HH, HH])
        w_h1 = load(w_h1_d, [HH, HH])
        w_out = load(w_out_d, [HH, 4 * HH])
        g_half = load(g_half_d, [HH, HH])
        g_full = load(g_full_d, [HH, HH])
        sw6 = load(sw6_d, [HH, H])
        sw3 = load(sw3_d, [HH, H])
        gw6 = load(gw6_d, [HH, HH])
        gw3 = load(gw3_d, [HH, HH])
        bias4 = load(bias4_d, [4, HH])
        ind4 = load(ind4_d, [4, 512])
        b_in = load(b_in_d, [HH, 1], F32)
        b_h0 = load(b_h0_d, [HH, 1], F32)
        b_h1 = load(b_h1_d, [HH, 1], F32)

        sb_h = ctx.enter_context(tc.tile_pool(name="h", bufs=4))
        sb_f = ctx.enter_context(tc.tile_pool(name="f", bufs=3))
        sb_p = ctx.enter_context(tc.tile_pool(name="prod", bufs=3))
        sb_z16 = ctx.enter_context(tc.tile_pool(name="z16p", bufs=2))
        sb_z32 = ctx.enter_context(tc.tile_pool(name="z32p", bufs=2))
        sb_d = ctx.enter_context(tc.tile_pool(name="d", bufs=6))
        ps_h = ctx.enter_context(tc.tile_pool(name="psh", bufs=2, space="PSUM"))
        ps_mix = ctx.enter_context(tc.tile_pool(name="psmix", bufs=1, space="PSUM"))
        ps_f = ctx.enter_context(tc.tile_pool(name="psf", bufs=1, space="PSUM"))

        z32 = [None, None]
        z16 = [None, None]
        for s in range(2):
            z32[s] = const.tile([H, SW], F32, tag=f"z32i{s}", name=f"z32i{s}")
            nc.sync.dma_start(z32[s][:], z32_d[s])
            z16[s] = const.tile([H, SW], F16, tag=f"z16i{s}", name=f"z16i{s}")
            nc.sync.dma_start(z16[s][:], z16_d[s])

        # Per-stream packed accumulator bank (ring of 2):
        #   cols 0:SW           -> pv: W_in^T z_next accumulator
        #   cols SW:2SW (64 prt)-> pz: sum_s w_s k_s accumulator
        mix = [None, None]
        prev_pv = [None, None]

        # initial pv = W_in^T z0
        for s in range(2):
            mix[s] = ps_mix.tile([128, 2 * SW], F32, tag=f"mix{s}",
                                 name=f"mixi{s}")
            nc.tensor.matmul(mix[s][:, 0:SW], w_in[:], z16[s][:],
                             start=True, stop=True)
            prev_pv[s] = mix[s][:, 0:SW]

        def ph_alloc(s):
            t = ps_h.tile([HH, SW], F32, tag=f"ph{s}", name=f"ph{s}")
            return t[:]

        for ti in range(nsteps):
            d_t = [None, None]
            for s in range(2):
                d_t[s] = sb_d.tile([128, 512], F16, tag=f"d{s}", name=f"d{s}")
                nc.sync.dma_start(d_t[s][:], drep_d[2 * ti + s])

            for s in range(2):
                mix[s] = ps_mix.tile([128, 2 * SW], F32, tag=f"mix{s}",
                                     name=f"mix{s}")
            pz = [mix[0][0:H, SW:2 * SW], mix[1][0:H, SW:2 * SW]]
            pv = [mix[0][:, 0:SW], mix[1][:, 0:SW]]
            ph1 = [prev_pv[0], prev_pv[1]]

            deferred = []
            for s4 in range(4):
                h1 = [None, None]
                for s in range(2):
                    h1[s] = sb_h.tile([HH, SW], F16, tag=f"h1{s}", name=f"h1{s}")
                    nc.vector.tensor_scalar(h1[s][:], ph1[s], b_in[:], 0.0,
                                            op0=ADD, op1=MAX)
                if s4 == 0:
                    # seed pv accumulator: W_in^T z_t (z16 of this step)
                    for s in range(2):
                        nc.tensor.matmul(pv[s], w_in[:], z16[s][:],
                                         start=True, stop=False,
                                         skip_group_check=True)
                ph2 = [None, None]
                for s in range(2):
                    ph2[s] = ph_alloc(s)
                    nc.tensor.matmul(ph2[s], w_h0[:], h1[s][:],
                                     start=True, stop=True)
                h2 = [None, None]
                for s in range(2):
                    h2[s] = sb_h.tile([HH, SW], F16, tag=f"h2{s}", name=f"h2{s}")
                    nc.scalar.activation(h2[s][:], ph2[s], RELU,
                                         bias=b_h0[:, 0:1])
                ph3 = [None, None]
                for s in range(2):
                    ph3[s] = ph_alloc(s)
                    nc.tensor.matmul(ph3[s], w_h1[:], h2[s][:],
                                     start=True, stop=True)
                h3 = [None, None]
                for s in range(2):
                    h3[s] = sb_h.tile([HH, SW], F16, tag=f"h3{s}", name=f"h3{s}")
                    nc.vector.tensor_scalar(h3[s][:], ph3[s], b_h1[:], 0.0,
                                            op0=ADD, op1=MAX)
                pf = [None, None]
                for s in range(2):
                    pf[s] = ps_f.tile([128, 512], F32, tag=f"pf{s}", name=f"pf{s}")
                    nc.tensor.matmul(pf[s][:], bias4[:], ind4[:],
                                     start=True, stop=False,
                                     skip_group_check=True)
                for s in range(2):
                    for j in range(4):
                        nc.tensor.matmul(pf[s][:, 128 * j:128 * (j + 1)],
                                         w_out[:, 128 * j:128 * (j + 1)],
                                         h3[s][:],
                                         start=False, stop=(j == 3),
                                         skip_group_check=True)
                f_sb = [None, None]
                for s in range(2):
                    f_sb[s] = sb_f.tile([128, 512], F16, tag=f"f{s}", name=f"f{s}")
                for s in range(2):
                    nc.scalar.activation(f_sb[s][:], pf[s][:], TANH)
                for emit in deferred:
                    emit()
                deferred = []
                prod = [None, None]
                for s in range(2):
                    prod[s] = sb_p.tile([128, 512], F16, tag=f"pr{s}", name=f"pr{s}")
                for s in range(2):
                    nc.vector.tensor_tensor(prod[s][:], f_sb[s][:],
                                            d_t[s][:], op=MULT)

                if s4 < 3:
                    g_mat = g_half if s4 < 2 else g_full
                    nxt = [None, None]
                    for s in range(2):
                        nxt[s] = ph_alloc(s)
                        nc.tensor.matmul(nxt[s], w_in[:], z16[s][:],
                                         start=True, stop=False,
                                         skip_group_check=True)
                    for s in range(2):
                        for j in range(4):
                            nc.tensor.matmul(nxt[s], g_mat[:],
                                             prod[s][:, 128 * j:128 * (j + 1)],
                                             start=False, stop=(j == 3),
                                             skip_group_check=True)
                    ph1 = [nxt[0], nxt[1]]
                else:
                    # boundary: pv must close before next step's relu1
                    for s in range(2):
                        for j in range(4):
                            nc.tensor.matmul(pv[s], gw6[:],
                                             prod[s][:, 128 * j:128 * (j + 1)],
                                             start=False, stop=(j == 3),
                                             skip_group_check=True)

                # off-chain accumulators: defer past the next stage's mm3 so
                # the in-order PE chews them inside the relu2/relu3 windows
                def make_folds(s4, prod):
                    def emit():
                        sw_mat = sw6 if s4 in (0, 3) else sw3
                        for s in range(2):
                            for j in range(4):
                                # the pv seed's start=True already zeroed
                                # this bank (incl. the pz region): accumulate
                                nc.tensor.matmul(
                                    pz[s], sw_mat[:],
                                    prod[s][:, 128 * j:128 * (j + 1)],
                                    start=False,
                                    stop=(s4 == 3 and j == 3),
                                    skip_group_check=True)
                        if s4 < 3:
                            gw_mat = gw6 if s4 in (0, 3) else gw3
                            for s in range(2):
                                for j in range(4):
                                    nc.tensor.matmul(
                                        pv[s], gw_mat[:],
                                        prod[s][:, 128 * j:128 * (j + 1)],
                                        start=False, stop=False,
                                        skip_group_check=True)
                    return emit

                if s4 < 3:
                    deferred.append(make_folds(s4, prod))
                else:
                    make_folds(s4, prod)()

            # step end (off the critical chain): z' = z + pz
            for s in range(2):
                z16_n = sb_z16.tile([H, SW], F16, tag=f"z16{s}", name=f"z16{s}")
                nc.vector.tensor_tensor(z16_n[:], pz[s], z32[s][:], op=ADD)
                z16[s] = z16_n
            for s in range(2):
                z32_n = sb_z32.tile([H, SW], F32, tag=f"z32{s}", name=f"z32{s}")
                nc.vector.tensor_tensor(z32_n[:], pz[s], z32[s][:], op=ADD)
                z32[s] = z32_n
            prev_pv = [pv[0], pv[1]]

        for s in range(2):
            nc.sync.dma_start(zT_d[s], z32[s][:])

    nc.compile()
    return nc


def _prep_inputs(coeffs, times, W_init, b_init, W_in, b_in, W_h, b_h,
                 W_out, b_out, nsteps):
    """Host-side constants + per-core shards."""
    coeffs = np.asarray(coeffs, np.float32)
    times = np.asarray(times, np.float32)
    dts_full = np.diff(times)
    dx = coeffs[:, 1:, :] - coeffs[:, :-1, :]
    dts = dts_full[:nsteps]
    dx = dx[:, :nsteps, :]
    dt0 = float(dts[0])
    assert np.allclose(dts, dt0, rtol=1e-4), "kernel assumes a uniform grid"

    z0 = coeffs[:, 0, :] @ np.asarray(W_init, np.float32) + np.asarray(b_init, np.float32)
    z0 = np.ascontiguousarray(z0.T)                      # [H, B] f32

    p = np.arange(128)
    j = np.arange(4)
    c_idx = 2 * j[None, :] + (p[:, None] >= 64)          # [128, 4]
    col = (p[:, None] % 64) * 8 + c_idx                  # [128, 4] output col

    W_in_f = np.asarray(W_in, np.float32)                # [H, HH]
    W_out = np.asarray(W_out, np.float32)                # [HH, 512]
    b_out = np.asarray(b_out, np.float32)                # [512]
    w_out_perm = np.ascontiguousarray(
        W_out[:, col.T.reshape(-1)]).astype(np.float16)  # [HH, (j,p) 512]
    bias4 = np.ascontiguousarray(b_out[col.T]).astype(np.float16)  # [4, 128]
    ind4 = np.kron(np.eye(4), np.ones((1, 128))).astype(np.float16)

    s_fold = (p[:, None] % 64 == np.arange(H)[None, :]).astype(np.float32)
    # G matrices: G_a[p, m] = a*dt * W_in[p%64, m]  (= a*dt * (S_fold @ W_in))
    g_base = s_fold @ W_in_f                             # [128, HH]
    g_half = (0.5 * g_base).astype(np.float16)
    g_full = (1.0 * g_base).astype(np.float16)
    sw6 = (s_fold / 6.0).astype(np.float16)
    sw3 = (s_fold / 3.0).astype(np.float16)
    gw6 = (g_base / 6.0).astype(np.float16)
    gw3 = (g_base / 3.0).astype(np.float16)

    W_h = np.asarray(W_h, np.float32)
    b_h = np.asarray(b_h, np.float32)
    consts = {
        "w_in": W_in_f.astype(np.float16),
        "w_h0": W_h[0].astype(np.float16),
        "w_h1": W_h[1].astype(np.float16),
        "w_out": w_out_perm,
        "g_half": g_half, "g_full": g_full,
        "sw6": sw6, "sw3": sw3,
        "gw6": gw6, "gw3": gw3,
        "bias4": bias4, "ind4": ind4,
        "b_in": np.asarray(b_in, np.float32).reshape(HH, 1).copy(),
        "b_h0": b_h[0].reshape(HH, 1).copy(),
        "b_h1": b_h[1].reshape(HH, 1).copy(),
    }

    in_maps = []
    for ci in range(N_CORES):
        bs, be = ci * BS, (ci + 1) * BS
        dx_t = dx[bs:be].transpose(1, 2, 0)              # [nsteps, C, 256]
        arr = dx_t.reshape(nsteps, 4, 2, 2, SW)          # [t, j, q, s, b]
        arr2 = arr.transpose(0, 3, 2, 1, 4).astype(np.float16)
        drep = np.broadcast_to(
            arr2[:, :, :, None, :, :],
            (nsteps, 2, 2, 64, 4, SW)).reshape(nsteps * 2, 128, 512)
        m = dict(consts)
        m["drep"] = np.ascontiguousarray(drep)
        zc = z0[:, bs:be]                                # [H, 256]
        m["z32"] = np.ascontiguousarray(
            zc.reshape(H, 2, SW).transpose(1, 0, 2))     # [2, H, SW]
        m["z16"] = m["z32"].astype(np.float16)
        in_maps.append(m)
    return in_maps, dts


_CACHE = {}


def _get_nc(nsteps, dts_key, dts):
    key = (nsteps, dts_key)
    if key not in _CACHE:
        _CACHE[key] = _build(nsteps, dts)
    return _CACHE[key]


def run_scan(coeffs, times, W_init, b_init, W_in, b_in, W_h, b_h, W_out, b_out,
             nsteps=None):
    """Run the device scan; returns zT [B, H] float32."""
    times = np.asarray(times, np.float32)
    if nsteps is None:
        nsteps = len(times) - 1
    in_maps, dts = _prep_inputs(coeffs, times, W_init, b_init, W_in, b_in,
                                W_h, b_h, W_out, b_out, nsteps)
    nc = _get_nc(nsteps, dts.tobytes(), dts)
    res = run_bass_kernel_spmd(nc, in_maps, core_ids=list(range(N_CORES)))
    outs = []
    for ci in range(N_CORES):
        zT = res.results[ci]["zT"]                       # [2, H, SW]
        outs.append(zT.transpose(1, 0, 2).reshape(H, BS))
    zT = np.concatenate(outs, axis=1)                    # [H, B]
    return np.ascontiguousarray(zT.T)


def kernel(coeffs, y, times, W_init, b_init, W_in, b_in, W_h, b_h,
           W_out, b_out, W_read, b_read):
    zT = run_scan(coeffs, times, W_init, b_init, W_in, b_in, W_h, b_h,
                  W_out, b_out)
    y = np.asarray(y)
    logits = (zT.astype(np.float64) @ np.asarray(W_read, np.float64)
              + np.asarray(b_read, np.float64))          # [B, O]
    m = logits.max(axis=1, keepdims=True)
    logp = logits - (m + np.log(np.exp(logits - m).sum(axis=1, keepdims=True)))
    loss = np.float32(-logp[np.arange(B), y].mean())
    acc = np.float32((logits.argmax(axis=1) == y).sum())
    return loss, acc


# revision 27
# speedup vs baseline: 1.0698x; 1.0652x over previous
"""Neural CDE forward pass on 8 Trainium2 NeuronCores.

Model (reference): z0 = coeffs[:,0]@W_init+b_init; RK4 scan over T-1=99 grid
intervals of dz = f(z) dX with f = MLP(64->128->128->128->512) -> tanh ->
reshape [H,C], contracted with dX/dt; then logits/loss/accuracy readout.

Sharding: pure data parallel over batch (2048 -> 8x256). Each core runs the
full scan on its shard; tiny readout done on host from the final z.

Device layout (per core): the 256-row batch shard is split into 2 independent
128-wide streams. All matmul operands are fp16 (1 cyc/row on PE); PSUM
accumulates fp32; the carried z state stays fp32. The wall-clock is set by
the serial RK4 dependency chain (396 stages), so the chain is shortened:

  - G-trick: the next stage's first-layer preactivation is computed directly
    in PSUM as  W_in^T z  (seed matmul)  +  a*dt*(S_fold W_in)^T prod
    (4 chunk matmuls), eliminating the fold->zs->mm1 round trip.
  - The RK4 combination  z' = z + sum_s w_s k_s  accumulates in a PSUM bank
    via 16 fold matmuls with pre-scaled stationaries (w_s * S_fold), read
    once per step.
  - [128->512] output layer: 4 column chunks (permuted so chunk j /
    partition p holds feature (h=p%64, c=2j+(p>=64))), bias via a K=4 seed
    matmul, tanh on Act, dX multiply on DVE (dX, not dX/dt: the dt factors
    are folded into the fold stationaries so all fp16 constants stay O(1)).
  - A second PSUM accumulator (pv) carries W_in^T z_{t+1} across the step
    boundary (seed + w_s-scaled G folds), so the step handoff is a normal
    stage edge; the fp32 z update runs off-chain on DVE.
  - pv and pz share one PSUM bank: a matmul with start=True resets the
    WHOLE bank (measured on HW), so the bank sees exactly one start per
    step (the pv seed, which also zeroes pz) and everything else
    accumulates with start=False.
  - The w_s/a_s-weighted fold trains are deferred past the next stage's
    mm3 emission so the in-order PE runs them inside the relu wait windows.
  - relu1/relu3 + prod on DVE, relu2 + tanh on Act (gpsimd cannot touch
    PSUM on TRN2).
"""

import numpy as np

from contextlib import ExitStack

from concourse import bacc, mybir
import concourse.tile as tile
from concourse.bass_utils import run_bass_kernel_spmd

N_CORES = 8
B, T, C, H, HH, O = 2048, 100, 8, 64, 128, 10
BS = B // N_CORES   # 256 batch rows per core
SW = 128            # stream width; 2 streams per core
F16 = mybir.dt.float16
F32 = mybir.dt.float32

ADD = mybir.AluOpType.add
MAX = mybir.AluOpType.max
MULT = mybir.AluOpType.mult
TANH = mybir.ActivationFunctionType.Tanh
RELU = mybir.ActivationFunctionType.Relu


def _build(nsteps, dts):
    """Build + compile the per-core Bass program. dts: python floats [nsteps],
    must be uniform (linspace grid)."""
    nc = bacc.Bacc("TRN2", target_bir_lowering=False, debug=False,
                   num_devices=N_CORES)

    def din(name, shape, dt=F16):
        return nc.dram_tensor(name, shape, dt, kind="ExternalInput").ap()

    z32_d = din("z32", [2, H, SW], F32)
    z16_d = din("z16", [2, H, SW], F16)
    drep_d = din("drep", [nsteps * 2, 128, 512], F16)
    w_in_d = din("w_in", [H, HH])
    w_h0_d = din("w_h0", [HH, HH])
    w_h1_d = din("w_h1", [HH, HH])
    w_out_d = din("w_out", [HH, 4 * HH])
    g_half_d = din("g_half", [HH, HH])
    g_full_d = din("g_full", [HH, HH])
    sw6_d = din("sw6", [HH, H])
    sw3_d = din("sw3", [HH, H])
    gw6_d = din("gw6", [HH, HH])
    gw3_d = din("gw3", [HH, HH])
    bias4_d = din("bias4", [4, HH])
    ind4_d = din("ind4", [4, 512])
    b_in_d = din("b_in", [HH, 1], F32)
    b_h0_d = din("b_h0", [HH, 1], F32)
    b_h1_d = din("b_h1", [HH, 1], F32)
    zT_d = nc.dram_tensor("zT", [2, H, SW], F32, kind="ExternalOutput").ap()

    with tile.TileContext(nc) as tc, ExitStack() as ctx:
        const = ctx.enter_context(tc.tile_pool(name="const", bufs=1))

        def load(ap_dram, shape, dt=F16):
            t = const.tile(shape, dt, tag=ap_dram.name, name=ap_dram.name)
            nc.sync.dma_start(t[:], ap_dram)
            return t

        w_in = load(w_in_d, [H, HH])
        w_h0 = load(w_h0_d, [HH, HH])
        w_h1 = load(w_h1_d, [HH, HH])
        w_out = load(w_out_d, [HH, 4 * HH])
        g_half = load(g_half_d, [HH, HH])
        g_full = load(g_full_d, [HH, HH])
        sw6 = load(sw6_d, [HH, H])
        sw3 = load(sw3_d, [HH, H])
        gw6 = load(gw6_d, [HH, HH])
        gw3 = load(gw3_d, [HH, HH])
        bias4 = load(bias4_d, [4, HH])
        ind4 = load(ind4_d, [4, 512])
        b_in = load(b_in_d, [HH, 1], F32)
        b_h0 = load(b_h0_d, [HH, 1], F32)
        b_h1 = load(b_h1_d, [HH, 1], F32)

        sb_h = ctx.enter_context(tc.tile_pool(name="h", bufs=4))
        sb_f = ctx.enter_context(tc.tile_pool(name="f", bufs=3))
        sb_p = ctx.enter_context(tc.tile_pool(name="prod", bufs=3))
        sb_z16 = ctx.enter_context(tc.tile_pool(name="z16p", bufs=2))
        sb_z32 = ctx.enter_context(tc.tile_pool(name="z32p", bufs=2))
        sb_d = ctx.enter_context(tc.tile_pool(name="d", bufs=6))
        ps_h = ctx.enter_context(tc.tile_pool(name="psh", bufs=2, space="PSUM"))
        ps_mix = ctx.enter_context(tc.tile_pool(name="psmix", bufs=1, space="PSUM"))
        ps_f = ctx.enter_context(tc.tile_pool(name="psf", bufs=1, space="PSUM"))

        z32 = [None, None]
        z16 = [None, None]
        for s in range(2):
            z32[s] = const.tile([H, SW], F32, tag=f"z32i{s}", name=f"z32i{s}")
            nc.sync.dma_start(z32[s][:], z32_d[s])
            z16[s] = const.tile([H, SW], F16, tag=f"z16i{s}", name=f"z16i{s}")
            nc.sync.dma_start(z16[s][:], z16_d[s])

        # Per-stream packed accumulator bank (ring of 2):
        #   cols 0:SW           -> pv: W_in^T z_next accumulator
        #   cols SW:2SW (64 prt)-> pz: sum_s w_s k_s accumulator
        mix = [None, None]
        prev_pv = [None, None]

        # initial pv = W_in^T z0
        for s in range(2):
            mix[s] = ps_mix.tile([128, 2 * SW], F32, tag=f"mix{s}",
                                 name=f"mixi{s}")
            nc.tensor.matmul(mix[s][:, 0:SW], w_in[:], z16[s][:],
                             start=True, stop=True)
            prev_pv[s] = mix[s][:, 0:SW]

        def ph_alloc(s):
            t = ps_h.tile([HH, SW], F32, tag=f"ph{s}", name=f"ph{s}")
            return t[:]

        pending_z = None
        for ti in range(nsteps):
            d_t = [None, None]
            for s in range(2):
                d_t[s] = sb_d.tile([128, 512], F16, tag=f"d{s}", name=f"d{s}")
                nc.sync.dma_start(d_t[s][:], drep_d[2 * ti + s])

            for s in range(2):
                mix[s] = ps_mix.tile([128, 2 * SW], F32, tag=f"mix{s}",
                                     name=f"mix{s}")
            pz = [mix[0][0:H, SW:2 * SW], mix[1][0:H, SW:2 * SW]]
            pv = [mix[0][:, 0:SW], mix[1][:, 0:SW]]
            ph1 = [prev_pv[0], prev_pv[1]]

            deferred = []
            for s4 in range(4):
                h1 = [None, None]
                for s in range(2):
                    h1[s] = sb_h.tile([HH, SW], F16, tag=f"h1{s}", name=f"h1{s}")
                    nc.vector.tensor_scalar(h1[s][:], ph1[s], b_in[:], 0.0,
                                            op0=ADD, op1=MAX)
                if s4 == 0 and pending_z is not None:
                    pending_z()
                    pending_z = None
                ph2 = [None, None]
                for s in range(2):
                    ph2[s] = ph_alloc(s)
                    nc.tensor.matmul(ph2[s], w_h0[:], h1[s][:],
                                     start=True, stop=True)
                h2 = [None, None]
                for s in range(2):
                    h2[s] = sb_h.tile([HH, SW], F16, tag=f"h2{s}", name=f"h2{s}")
                    nc.scalar.activation(h2[s][:], ph2[s], RELU,
                                         bias=b_h0[:, 0:1])
                ph3 = [None, None]
                for s in range(2):
                    ph3[s] = ph_alloc(s)
                    nc.tensor.matmul(ph3[s], w_h1[:], h2[s][:],
                                     start=True, stop=True)
                h3 = [None, None]
                for s in range(2):
                    h3[s] = sb_h.tile([HH, SW], F16, tag=f"h3{s}", name=f"h3{s}")
                    nc.vector.tensor_scalar(h3[s][:], ph3[s], b_h1[:], 0.0,
                                            op0=ADD, op1=MAX)
                pf = [None, None]
                for s in range(2):
                    pf[s] = ps_f.tile([128, 512], F32, tag=f"pf{s}", name=f"pf{s}")
                    nc.tensor.matmul(pf[s][:], bias4[:], ind4[:],
                                     start=True, stop=False,
                                     skip_group_check=True)
                for s in range(2):
                    for j in range(4):
                        nc.tensor.matmul(pf[s][:, 128 * j:128 * (j + 1)],
                                         w_out[:, 128 * j:128 * (j + 1)],
                                         h3[s][:],
                                         start=False, stop=(j == 3),
                                         skip_group_check=True)
                if s4 == 0:
                    # seed pv accumulator: W_in^T z_t. Emitted after wout so
                    # its wait on the deferred z16 cannot stall mm2/mm3 on
                    # the in-order PE; it only has to precede the stage-1
                    # fold flush.
                    for s in range(2):
                        nc.tensor.matmul(pv[s], w_in[:], z16[s][:],
                                         start=True, stop=False,
                                         skip_group_check=True)
                f_sb = [None, None]
                for s in range(2):
                    f_sb[s] = sb_f.tile([128, 512], F16, tag=f"f{s}", name=f"f{s}")
                for s in range(2):
                    nc.scalar.activation(f_sb[s][:], pf[s][:], TANH)
                for emit in deferred:
                    emit()
                deferred = []
                prod = [None, None]
                for s in range(2):
                    prod[s] = sb_p.tile([128, 512], F16, tag=f"pr{s}", name=f"pr{s}")
                for s in range(2):
                    nc.vector.tensor_tensor(prod[s][:], f_sb[s][:],
                                            d_t[s][:], op=MULT)

                if s4 < 3:
                    g_mat = g_half if s4 < 2 else g_full
                    nxt = [None, None]
                    for s in range(2):
                        nxt[s] = ph_alloc(s)
                        nc.tensor.matmul(nxt[s], w_in[:], z16[s][:],
                                         start=True, stop=False,
                                         skip_group_check=True)
                    for s in range(2):
                        for j in range(4):
                            nc.tensor.matmul(nxt[s], g_mat[:],
                                             prod[s][:, 128 * j:128 * (j + 1)],
                                             start=False, stop=(j == 3),
                                             skip_group_check=True)
                    ph1 = [nxt[0], nxt[1]]
                else:
                    # boundary: pv must close before next step's relu1
                    for s in range(2):
                        for j in range(4):
                            nc.tensor.matmul(pv[s], gw6[:],
                                             prod[s][:, 128 * j:128 * (j + 1)],
                                             start=False, stop=(j == 3),
                                             skip_group_check=True)

                # off-chain accumulators: defer past the next stage's mm3 so
                # the in-order PE chews them inside the relu2/relu3 windows
                def make_folds(s4, prod):
                    def emit():
                        sw_mat = sw6 if s4 in (0, 3) else sw3
                        for s in range(2):
                            for j in range(4):
                                # the pv seed's start=True already zeroed
                                # this bank (incl. the pz region): accumulate
                                nc.tensor.matmul(
                                    pz[s], sw_mat[:],
                                    prod[s][:, 128 * j:128 * (j + 1)],
                                    start=False,
                                    stop=(s4 == 3 and j == 3),
                                    skip_group_check=True)
                        if s4 < 3:
                            gw_mat = gw6 if s4 in (0, 3) else gw3
                            for s in range(2):
                                for j in range(4):
                                    nc.tensor.matmul(
                                        pv[s], gw_mat[:],
                                        prod[s][:, 128 * j:128 * (j + 1)],
                                        start=False, stop=False,
                                        skip_group_check=True)
                    return emit

                if s4 < 3:
                    deferred.append(make_folds(s4, prod))
                else:
                    make_folds(s4, prod)()

            # step end z' = z + pz: deferred into the next step's stage 0
            # (behind relu1 in DVE order) so it does not delay the chain
            def mk_pending(pz=pz, z32_old=list(z32)):
                def emit():
                    for s in range(2):
                        z16_n = sb_z16.tile([H, SW], F16, tag=f"z16{s}",
                                            name=f"z16{s}")
                        nc.vector.tensor_tensor(z16_n[:], pz[s],
                                                z32_old[s][:], op=ADD)
                        z16[s] = z16_n
                    for s in range(2):
                        z32_n = sb_z32.tile([H, SW], F32, tag=f"z32{s}",
                                            name=f"z32{s}")
                        nc.vector.tensor_tensor(z32_n[:], pz[s],
                                                z32_old[s][:], op=ADD)
                        z32[s] = z32_n
                return emit
            pending_z = mk_pending()
            prev_pv = [pv[0], pv[1]]

        pending_z()
        for s in range(2):
            nc.sync.dma_start(zT_d[s], z32[s][:])

    nc.compile()
    return nc


def _prep_inputs(coeffs, times, W_init, b_init, W_in, b_in, W_h, b_h,
                 W_out, b_out, nsteps):
    """Host-side constants + per-core shards."""
    coeffs = np.asarray(coeffs, np.float32)
    times = np.asarray(times, np.float32)
    dts_full = np.diff(times)
    dx = coeffs[:, 1:, :] - coeffs[:, :-1, :]
    dts = dts_full[:nsteps]
    dx = dx[:, :nsteps, :]
    dt0 = float(dts[0])
    assert np.allclose(dts, dt0, rtol=1e-4), "kernel assumes a uniform grid"

    z0 = coeffs[:, 0, :] @ np.asarray(W_init, np.float32) + np.asarray(b_init, np.float32)
    z0 = np.ascontiguousarray(z0.T)                      # [H, B] f32

    p = np.arange(128)
    j = np.arange(4)
    c_idx = 2 * j[None, :] + (p[:, None] >= 64)          # [128, 4]
    col = (p[:, None] % 64) * 8 + c_idx                  # [128, 4] output col

    W_in_f = np.asarray(W_in, np.float32)                # [H, HH]
    W_out = np.asarray(W_out, np.float32)                # [HH, 512]
    b_out = np.asarray(b_out, np.float32)                # [512]
    w_out_perm = np.ascontiguousarray(
        W_out[:, col.T.reshape(-1)]).astype(np.float16)  # [HH, (j,p) 512]
    bias4 = np.ascontiguousarray(b_out[col.T]).astype(np.float16)  # [4, 128]
    ind4 = np.kron(np.eye(4), np.ones((1, 128))).astype(np.float16)

    s_fold = (p[:, None] % 64 == np.arange(H)[None, :]).astype(np.float32)
    # G matrices: G_a[p, m] = a*dt * W_in[p%64, m]  (= a*dt * (S_fold @ W_in))
    g_base = s_fold @ W_in_f                             # [128, HH]
    g_half = (0.5 * g_base).astype(np.float16)
    g_full = (1.0 * g_base).astype(np.float16)
    sw6 = (s_fold / 6.0).astype(np.float16)
    sw3 = (s_fold / 3.0).astype(np.float16)
    gw6 = (g_base / 6.0).astype(np.float16)
    gw3 = (g_base / 3.0).astype(np.float16)

    W_h = np.asarray(W_h, np.float32)
    b_h = np.asarray(b_h, np.float32)
    consts = {
        "w_in": W_in_f.astype(np.float16),
        "w_h0": W_h[0].astype(np.float16),
        "w_h1": W_h[1].astype(np.float16),
        "w_out": w_out_perm,
        "g_half": g_half, "g_full": g_full,
        "sw6": sw6, "sw3": sw3,
        "gw6": gw6, "gw3": gw3,
        "bias4": bias4, "ind4": ind4,
        "b_in": np.asarray(b_in, np.float32).reshape(HH, 1).copy(),
        "b_h0": b_h[0].reshape(HH, 1).copy(),
        "b_h1": b_h[1].reshape(HH, 1).copy(),
    }

    in_maps = []
    for ci in range(N_CORES):
        bs, be = ci * BS, (ci + 1) * BS
        dx_t = dx[bs:be].transpose(1, 2, 0)              # [nsteps, C, 256]
        arr = dx_t.reshape(nsteps, 4, 2, 2, SW)          # [t, j, q, s, b]
        arr2 = arr.transpose(0, 3, 2, 1, 4).astype(np.float16)
        drep = np.broadcast_to(
            arr2[:, :, :, None, :, :],
            (nsteps, 2, 2, 64, 4, SW)).reshape(nsteps * 2, 128, 512)
        m = dict(consts)
        m["drep"] = np.ascontiguousarray(drep)
        zc = z0[:, bs:be]                                # [H, 256]
        m["z32"] = np.ascontiguousarray(
            zc.reshape(H, 2, SW).transpose(1, 0, 2))     # [2, H, SW]
        m["z16"] = m["z32"].astype(np.float16)
        in_maps.append(m)
    return in_maps, dts


_CACHE = {}


def _get_nc(nsteps, dts_key, dts):
    key = (nsteps, dts_key)
    if key not in _CACHE:
        _CACHE[key] = _build(nsteps, dts)
    return _CACHE[key]


def run_scan(coeffs, times, W_init, b_init, W_in, b_in, W_h, b_h, W_out, b_out,
             nsteps=None):
    """Run the device scan; returns zT [B, H] float32."""
    times = np.asarray(times, np.float32)
    if nsteps is None:
        nsteps = len(times) - 1
    in_maps, dts = _prep_inputs(coeffs, times, W_init, b_init, W_in, b_in,
                                W_h, b_h, W_out, b_out, nsteps)
    nc = _get_nc(nsteps, dts.tobytes(), dts)
    res = run_bass_kernel_spmd(nc, in_maps, core_ids=list(range(N_CORES)))
    outs = []
    for ci in range(N_CORES):
        zT = res.results[ci]["zT"]                       # [2, H, SW]
        outs.append(zT.transpose(1, 0, 2).reshape(H, BS))
    zT = np.concatenate(outs, axis=1)                    # [H, B]
    return np.ascontiguousarray(zT.T)


def kernel(coeffs, y, times, W_init, b_init, W_in, b_in, W_h, b_h,
           W_out, b_out, W_read, b_read):
    zT = run_scan(coeffs, times, W_init, b_init, W_in, b_in, W_h, b_h,
                  W_out, b_out)
    y = np.asarray(y)
    logits = (zT.astype(np.float64) @ np.asarray(W_read, np.float64)
              + np.asarray(b_read, np.float64))          # [B, O]
    m = logits.max(axis=1, keepdims=True)
    logp = logits - (m + np.log(np.exp(logits - m).sum(axis=1, keepdims=True)))
    loss = np.float32(-logp[np.arange(B), y].mean())
    acc = np.float32((logits.argmax(axis=1) == y).sum())
    return loss, acc
